# revision 20
# baseline (speedup 1.0000x reference)
"""Causal self-attention (B=2, S=2048, D=1024, H=16) on 8 TRN2 NeuronCores.

Sharding: tensor-parallel over heads (2 heads/core) for qkv+attention,
then AllToAll to token-parallel (512 tokens/core) for the output projection.

Per-core kernel (SPMD, identical program, per-core weight slices as inputs):
  1. qkv^T projection from host-pre-transposed, host-bf16-cast x^T:
       Q^T/K^T/V^T [128ch(2 heads x 64), 4096 tok] = W_slice^T @ x
     (bf16 matmuls, fp32 PSUM accumulation)
  2. V^T -> V via PE transposes (dense block; transposes don't count as
     HAM activity, so they must not be sprinkled through attention),
     ones column appended so AV also produces the softmax denominator.
  3. Per (batch, head): S^T tiles (keys on partitions), exp on ScalarE with
     scale=1/sqrt(hd) fused, causal = block skipping + one triangle mask
     multiply on diagonal blocks; unnormalized y^T + denominator row
     accumulate in PSUM and ship to DRAM in fp32.
  4. AllToAll (fp32, ~2MB/rank): head-slices -> full channels for my tokens.
  5. Scale each received head-chunk by its reciprocal denominator (fused
     into the f32->bf16 cast), then out[my 512 tokens] = y^T.T @ W_proj + b.
Host gathers the 8 token-slices and reshapes.
"""

import numpy as np
from contextlib import ExitStack

import concourse.bass as bass
import concourse.bacc as bacc
import concourse.tile as tile
from concourse import mybir
from concourse.bass_utils import run_bass_kernel_spmd
from concourse.masks import make_identity

B, S, D = 2, 2048, 1024
H, HD = 16, 64
NCORE = 8
HPC = H // NCORE          # heads per core = 2
CW = HPC * HD             # channels per core = 128
T = B * S                 # 4096 tokens
TPC = T // NCORE          # 512 tokens per core (proj phase)
TCH = 512                 # token chunk for qkv projection
NT = T // TCH             # 8
QCH = 512                 # query chunk
KCH = 128                 # key chunk
NQC = S // QCH            # 4 query chunks per batch
DK = D // 128             # 8 contraction chunks of 128

f32 = mybir.dt.float32
f32r = mybir.dt.float32r
bf16 = mybir.dt.bfloat16
AF = mybir.ActivationFunctionType


def _build():
    nc = bacc.Bacc(None, target_bir_lowering=False, num_devices=NCORE)

    xT = nc.dram_tensor("xT", [D, T], bf16, kind="ExternalInput")
    wq = nc.dram_tensor("wq", [D, CW], bf16, kind="ExternalInput")
    wk = nc.dram_tensor("wk", [D, CW], bf16, kind="ExternalInput")
    wv = nc.dram_tensor("wv", [D, CW], bf16, kind="ExternalInput")
    bqkv = nc.dram_tensor("bqkv", [3, CW], f32, kind="ExternalInput")
    wp = nc.dram_tensor("wp", [D, D], bf16, kind="ExternalInput")
    bp = nc.dram_tensor("bp", [1, D], f32, kind="ExternalInput")
    out = nc.dram_tensor("out", [TPC, D], f32, kind="ExternalOutput")

    with ExitStack() as ctx:
        tc = ctx.enter_context(tile.TileContext(nc))
        const = ctx.enter_context(tc.tile_pool(name="const", bufs=1))
        dram = ctx.enter_context(tc.tile_pool(name="dram", bufs=1, space="DRAM"))
        wqkv_pool = ctx.enter_context(tc.tile_pool(name="wqkv", bufs=1))
        xt_pool = ctx.enter_context(tc.tile_pool(name="xt", bufs=3))
        qkvt_pool = ctx.enter_context(tc.tile_pool(name="qkvt", bufs=1))
        wp_pool = ctx.enter_context(tc.tile_pool(name="wpp", bufs=1))
        vpool = ctx.enter_context(tc.tile_pool(name="vpool", bufs=4))
        ppool = ctx.enter_context(tc.tile_pool(name="ppool", bufs=9))
        ynpool = ctx.enter_context(tc.tile_pool(name="ynpool", bufs=3))
        recv_pool = ctx.enter_context(tc.tile_pool(name="recvp", bufs=1))
        opool = ctx.enter_context(tc.tile_pool(name="opool", bufs=2))
        ps_big = ctx.enter_context(tc.tile_pool(name="ps_big", bufs=2, space="PSUM"))
        ps_sc = ctx.enter_context(tc.tile_pool(name="ps_sc", bufs=2, space="PSUM"))
        ps_y = ctx.enter_context(tc.tile_pool(name="ps_y", bufs=4, space="PSUM"))

        # ---- constants ----
        identity = const.tile([128, 128], bf16)
        make_identity(nc, identity[:])
        # mask[k, q] = 1.0 if k <= q else 0.0  (keep lower-left in S^T layout)
        mask = const.tile([128, 128], bf16)
        nc.gpsimd.memset(mask[:], 0.0)
        nc.gpsimd.affine_select(
            out=mask[:], in_=mask[:],
            compare_op=mybir.AluOpType.is_ge,  # iota(k-q-1) >= 0 (k>q) -> keep 0; else fill 1
            fill=1.0, base=-1, pattern=[[-1, 128]], channel_multiplier=1,
        )
        ones_f32 = const.tile([128, 128], f32)
        nc.vector.memset(ones_f32[:], 1.0)
        ones_bf = const.tile([128, 16], bf16)
        nc.vector.memset(ones_bf[:], 1.0)
        ones_row = const.tile([1, 128], f32r)
        nc.vector.tensor_copy(ones_row[:], ones_f32[0:1, :])
        bias_sb = const.tile([128, 3], f32)
        nc.sync.dma_start(bias_sb[:], bqkv[:].rearrange("g p -> p g"))
        bp_sb = const.tile([1, D], f32r)
        nc.sync.dma_start(bp_sb[:], bp[:].bitcast(f32r))

        # ---- weights (bf16) ----
        wq_sb = wqkv_pool.tile([128, DK, CW], bf16)
        wk_sb = wqkv_pool.tile([128, DK, CW], bf16)
        wv_sb = wqkv_pool.tile([128, DK, CW], bf16)
        nc.sync.dma_start(wq_sb[:], wq[:].rearrange("(c p) m -> p c m", p=128))
        nc.sync.dma_start(wk_sb[:], wk[:].rearrange("(c p) m -> p c m", p=128))
        nc.sync.dma_start(wv_sb[:], wv[:].rearrange("(c p) m -> p c m", p=128))
        wp_sb = wp_pool.tile([128, DK, D], bf16)
        nc.sync.dma_start(wp_sb[:], wp[:].rearrange("(c p) m -> p c m", p=128))

        # ---- A2A buffers (fp32) ----
        SH = CW + 2  # shard rows: 128 channels + one denominator row per local head
        send_t = dram.tile([NCORE, SH, TPC], f32)
        recv_t = dram.tile([NCORE, SH, TPC], f32)

        # ---- phase 1: qkv^T projection ----
        qT = qkvt_pool.tile([128, T], bf16)
        kT = qkvt_pool.tile([128, T], bf16)
        vT = qkvt_pool.tile([128, T], bf16)
        for tp in range(NT // 2):
            t0, t1 = 2 * tp, 2 * tp + 1
            xt = xt_pool.tile([128, DK, 2 * TCH], bf16, tag="xt")
            nc.sync.dma_start(
                xt[:],
                xT[:].rearrange("(c p) t -> p c t", p=128)[:, :, t0 * TCH:(t1 + 1) * TCH],
            )
            for gi, (wsb, dst) in enumerate([(wq_sb, qT), (wk_sb, kT), (wv_sb, vT)]):
                ps0 = ps_big.tile([128, TCH], f32, tag="psbig")
                ps1 = ps_big.tile([128, TCH], f32, tag="psbig")
                for c in range(DK):
                    # consecutive matmuls share lhsT -> one weight load serves two
                    nc.tensor.matmul(
                        ps0[:], lhsT=wsb[:, c, :], rhs=xt[:, c, 0:TCH],
                        start=(c == 0), stop=(c == DK - 1),
                    )
                    nc.tensor.matmul(
                        ps1[:], lhsT=wsb[:, c, :], rhs=xt[:, c, TCH:2 * TCH],
                        start=(c == 0), stop=(c == DK - 1),
                    )
                for ti, ps in ((t0, ps0), (t1, ps1)):
                    nc.scalar.activation(
                        dst[:, ti * TCH:(ti + 1) * TCH], ps[:], AF.Identity,
                        bias=bias_sb[:, gi:gi + 1],
                    )

        # ---- phase 2: all V transposes in one dense block (PE transposes are
        # HAM-invisible; sprinkling them through attention keeps the clock cold) ----
        vlocs = {}
        for b in range(B):
            for hl in range(HPC):
                r0 = hl * HD
                vloc = vpool.tile([128, (S // KCH) * (HD + 1)], bf16, tag="v",
                                  name=f"vloc_{b}_{hl}")
                vlocs[(b, hl)] = vloc
                nc.vector.tensor_copy(vloc[:, HD::HD + 1], ones_bf[:, 0:S // KCH])
                VG = 8  # transposes batched per PSUM bank
                for g in range(S // KCH // VG):
                    pst = ps_big.tile([128, VG, HD], bf16, tag="psbig")
                    for u in range(VG):
                        kc = g * VG + u
                        nc.tensor.transpose(
                            pst[:, u, :],
                            vT[r0:r0 + HD, b * S + kc * KCH: b * S + (kc + 1) * KCH],
                            identity[r0:r0 + HD, r0:r0 + HD],
                        )
                    nc.vector.tensor_copy(
                        vloc[:].rearrange("p (c w) -> p c w", w=HD + 1)[:, g * VG:(g + 1) * VG, 0:HD],
                        pst[:],
                    )

        # ---- phase 3: attention, kc-outer over query-chunk pairs; the two
        # heads' scores matmuls sit in disjoint PE row groups (0-63 / 64-127)
        # and are emitted adjacently so they execute concurrently ----
        NKC = S // KCH  # 16
        for b in range(B):
            for qh in range(NQC // 2):
                qcs = (2 * qh, 2 * qh + 1)
                ypss = {(qc, hl): ps_y.tile([HD + 1, QCH], f32, tag="yps",
                                            name=f"yps{b}{qc}{hl}")
                        for qc in qcs for hl in range(HPC)}

                def geom(qc, kc):
                    q0 = qc * QCH
                    diag = kc >= (q0 // KCH)
                    koff = kc * KCH - q0 if diag else 0
                    return koff, QCH - koff

                nkc_half = 4 * (qcs[-1] + 1)

                def qcs_for(kc):
                    return [qc for qc in qcs if kc * KCH < (qc + 1) * QCH]

                pts = {}

                def emit_scores_group(kc):
                    for qc in qcs_for(kc):
                        koff, W_ = geom(qc, kc)
                        for hl in range(HPC):
                            r0 = hl * HD
                            sps = ps_sc.tile([128, QCH], f32, tag="sps")
                            nc.tensor.matmul(
                                sps[:, 0:W_],
                                lhsT=kT[r0:r0 + HD, b * S + kc * KCH: b * S + (kc + 1) * KCH],
                                rhs=qT[r0:r0 + HD, b * S + qc * QCH + koff: b * S + (qc + 1) * QCH],
                                start=True, stop=True,
                            )
                            pt = ppool.tile([128, QCH], bf16, tag="pt")
                            nc.scalar.activation(pt[:, 0:W_], sps[:, 0:W_], AF.Exp, scale=0.125)
                            if kc * KCH >= qc * QCH:
                                nc.vector.tensor_mul(pt[:, 0:KCH], pt[:, 0:KCH], mask[:])
                            pts[(qc, kc, hl)] = pt

                def emit_av_group(kc):
                    for qc in qcs_for(kc):
                        koff, W_ = geom(qc, kc)
                        last = (kc + 1) * KCH >= (qc + 1) * QCH
                        for hl in range(HPC):
                            pt = pts.pop((qc, kc, hl))
                            nc.tensor.matmul(
                                ypss[(qc, hl)][:, koff:QCH],
                                lhsT=vlocs[(b, hl)][:, kc * (HD + 1):(kc + 1) * (HD + 1)],
                                rhs=pt[:, 0:W_],
                                start=(kc == 0), stop=last,
                            )
                        if last:
                            for hl in range(HPC):
                                ysb = ynpool.tile([HD + 1, QCH], f32, tag="ysb")
                                nc.scalar.copy(ysb[:], ypss[(qc, hl)][:])
                                j = b * NQC + qc
                                nc.sync.dma_start(
                                    send_t[j, hl * HD:(hl + 1) * HD, :], ysb[0:HD, :])
                                nc.sync.dma_start(
                                    send_t[j, CW + hl, :], ysb[HD:HD + 1, :])

                emit_scores_group(0)
                for kc in range(nkc_half):
                    if kc + 1 < nkc_half:
                        emit_scores_group(kc + 1)
                    emit_av_group(kc)

        # ---- phase 4: AllToAll ----
        nc.gpsimd.collective_compute(
            "AllToAll", mybir.AluOpType.bypass,
            replica_groups=[list(range(NCORE))],
            ins=[send_t[:].opt()], outs=[recv_t[:].opt()],
        )

        # ---- phase 5: output projection for my 512 tokens ----
        rsb32 = recv_pool.tile([128, NCORE, TPC], f32)
        nc.sync.dma_start(rsb32[:], recv_t[:, 0:CW, :].rearrange("h p t -> p h t"))
        lrows = recv_pool.tile([2 * NCORE, TPC], f32)
        nc.sync.dma_start(lrows[0:NCORE, :], recv_t[:, CW, :])
        nc.sync.dma_start(lrows[NCORE:2 * NCORE, :], recv_t[:, CW + 1, :])
        lrec = recv_pool.tile([2 * NCORE, TPC], f32)
        nc.vector.reciprocal(lrec[:], lrows[:])
        lrecr = recv_pool.tile([2 * NCORE, TPC], f32r)
        nc.vector.tensor_copy(lrecr[:], lrec[:])
        rsb = recv_pool.tile([128, NCORE, TPC], bf16)
        # scale each 64-row half-chunk by its head's reciprocal denominator,
        # fused into the f32 -> bf16 cast
        for k in range(NCORE):
            for hl2 in range(2):
                lst = opool.tile([1, TPC], f32r, tag="lst", bufs=4)
                nc.sync.dma_start(lst[:], lrecr[hl2 * NCORE + k:hl2 * NCORE + k + 1, :])
                bc = ps_sc.tile([HD, TPC], f32, tag="sps")
                nc.tensor.matmul(
                    bc[:], lhsT=ones_row[:, 0:HD], rhs=lst[:],
                    start=True, stop=True,
                )
                nc.vector.tensor_mul(
                    rsb[hl2 * HD:(hl2 + 1) * HD, k, :],
                    rsb32[hl2 * HD:(hl2 + 1) * HD, k, :], bc[:])
        for m in range(TPC // 128):
            pss = [ps_big.tile([128, 512], f32, tag="psbig", name=f"pso{m}{n}")
                   for n in range(D // 512)]
            for c in range(DK):
                for n in range(D // 512):
                    nc.tensor.matmul(
                        pss[n][:],
                        lhsT=rsb[:, c, m * 128:(m + 1) * 128],
                        rhs=wp_sb[:, c, n * 512:(n + 1) * 512],
                        start=(c == 0), stop=False,
                    )
            for n in range(D // 512):
                nc.tensor.matmul(
                    pss[n][:], lhsT=ones_row[:],
                    rhs=bp_sb[:, n * 512:(n + 1) * 512],
                    start=False, stop=True,
                )
                osb = opool.tile([128, 512], f32, tag="osb")
                nc.scalar.copy(osb[:], pss[n][:])
                nc.sync.dma_start(out[m * 128:(m + 1) * 128, n * 512:(n + 1) * 512], osb[:])

    nc.compile()
    return nc


_NC_CACHE = None


def _get_nc():
    global _NC_CACHE
    if _NC_CACHE is None:
        _NC_CACHE = _build()
    return _NC_CACHE


def _bf16(a):
    import ml_dtypes
    return np.ascontiguousarray(a.astype(ml_dtypes.bfloat16))


def _in_maps(x, W_attn, b_attn, W_proj, b_proj):
    x = np.ascontiguousarray(np.asarray(x, dtype=np.float32))
    W_attn = np.asarray(W_attn, dtype=np.float32)
    b_attn = np.asarray(b_attn, dtype=np.float32)
    W_proj = np.ascontiguousarray(np.asarray(W_proj, dtype=np.float32))
    b_proj = np.asarray(b_proj, dtype=np.float32)

    xT = _bf16(x.reshape(T, D).T)  # [D, T] bf16
    wp16 = _bf16(W_proj)
    bp2 = np.ascontiguousarray(b_proj.reshape(1, D))
    maps = []
    for c in range(NCORE):
        lo = c * CW
        sl_q = slice(lo, lo + CW)
        sl_k = slice(D + lo, D + lo + CW)
        sl_v = slice(2 * D + lo, 2 * D + lo + CW)
        maps.append({
            "xT": xT,
            "wq": _bf16(W_attn[:, sl_q]),
            "wk": _bf16(W_attn[:, sl_k]),
            "wv": _bf16(W_attn[:, sl_v]),
            "bqkv": np.ascontiguousarray(
                np.stack([b_attn[sl_q], b_attn[sl_k], b_attn[sl_v]])),
            "wp": wp16,
            "bp": bp2,
        })
    return maps


def _gather(results):
    outs = [np.asarray(r["out"]) for r in results]
    return np.concatenate(outs, axis=0).reshape(B, S, D)


def kernel(x, W_attn, b_attn, W_proj, b_proj):
    nc = _get_nc()
    maps = _in_maps(x, W_attn, b_attn, W_proj, b_proj)
    res = run_bass_kernel_spmd(nc, maps, core_ids=list(range(NCORE)))
    return _gather(res.results)


def kernel_traced(x, W_attn, b_attn, W_proj, b_proj, **kw):
    """Same as kernel() but with NTFF tracing; returns (out, BassKernelResults)."""
    nc = _get_nc()
    maps = _in_maps(x, W_attn, b_attn, W_proj, b_proj)
    res = run_bass_kernel_spmd(nc, maps, core_ids=list(range(NCORE)), trace=True, **kw)
    return _gather(res.results), res
